# revision 16
# baseline (speedup 1.0000x reference)
"""DTW layer (short kernel) Trainium2 Bass kernel.

Problem: x (B=8, C=8, L=4096) f32, kernels (F=32, K=10) f32.
For each (b, c, f, w): DTW cost between kernels[f] (len 10) and window
x[b, c, 5w : 5w+20], for w in [0, 815). Output (B, C*F, 815) f32.

Sharding: data-parallel over batch — core b computes batch b entirely
(C*F = 256 (c,f) combos = 2 partition chunks of 128).

Algorithm (per core): the DTW row recurrence
    row_i[j] = D[i,j] + min(row_i[j-1], row_{i-1}[j], row_{i-1}[j-1])
is computed for 128 (c,f) combos at once (partition dim) and a chunk of
windows laid out along the free dim as [w, 21] segments (1 separator +
20 cells).  Per row:
  - ACT computes local costs D[w, 1+j] = (x[5w+j] - k_i)^2 via
    activation(Square, bias=-k_i) with an overlapping strided input AP.
  - DVE computes m[t] = min(S_prev[t], S_prev[t-1]) (one shifted min).
  - DVE tensor_tensor_scan: state = min(m[t], state) + D[t] computes the
    whole row for all windows in one instruction.  A BIG value in the
    separator column of D forces the carry to BIG between windows, which
    the min against m (= prev row values) then discards — resetting the
    recurrence at each window boundary.

Raw bass (no Tile framework): this toolchain's walrus codegen allows at
most 2 embedded sync-waits per instruction and rejects Tile's tail
drain, so engines are programmed directly with standalone wait_ge
instructions and three semaphores (dma/act/dve).
"""

from contextlib import ExitStack

import numpy as np

import concourse.bass as bass
import concourse.mybir as mybir
from concourse.bass_utils import run_bass_kernel_spmd

# Problem constants (hardcoded per harness contract)
B, C, L = 8, 8, 4096
F, K = 32, 10
PROC, STEP = 20, 5
NW = 815          # windows actually computed == chan_outlen
SEG = PROC + 1    # 1 separator + 20 cells
NWC = 136         # windows per chunk; 6 chunks = 816 >= 815
NCHUNK = 6
TFREE = NWC * SEG # 2856 scan length
BIG = 1e30
SLOTS = 2
UNITS = [(cc, wc) for cc in range(2) for wc in range(NCHUNK)]

F32 = mybir.dt.float32


def _build_nc(reps: int = 1) -> bass.Bass:
    # detect_race_conditions=False: CoreSim's detector does not model
    # same-engine program order, which this kernel relies on throughout.
    # reps > 1 replicates the whole schedule (for slope-based timing).
    nc = bass.Bass("TRN2", debug=False, detect_race_conditions=False)
    x_d = nc.dram_tensor("x", [C, L], F32, kind="ExternalInput").ap()
    k_d = nc.dram_tensor("negk", [F, K], F32, kind="ExternalInput").ap()
    out_d = nc.dram_tensor("out", [C * F, NWC * NCHUNK], F32,
                           kind="ExternalOutput").ap()

    UNITS_R = UNITS * reps

    # --- semaphore bookkeeping (python-side counts) ---
    # DVE op order: 8 init memsets, then per unit: scan0, (m,scan)x9 = 19
    def dve_through_scan(u, i):
        return 8 + 19 * u + (1 if i == 0 else 2 * i + 1)

    # ACT op order: per unit: 10 squares + 1 extract copy
    def act_through_square(u, i):
        return 11 * u + i + 1

    def act_through_copy(u):
        return 11 * (u + 1)

    def dma_through_out(u):  # 3 init DMAs then one out-DMA per unit
        return 16 * (4 + u)

    with ExitStack() as ctx:
        sb = lambda shape, name: ctx.enter_context(
            nc.sbuf_tensor(name, shape, F32))
        X = [sb([128, L], f"Xt{cc}") for cc in range(2)]
        negK = sb([128, K], "negKt")
        m0 = sb([128, TFREE], "m0t")
        S = [[sb([128, TFREE], f"St{s}_{i}") for i in range(2)]
             for s in range(SLOTS)]
        M = [sb([128, TFREE], f"Mt{s}") for s in range(SLOTS)]
        D = [[sb([128, TFREE], f"Dt{s}_{i}") for i in range(2)]
             for s in range(SLOTS)]
        OB = [sb([128, NWC], f"OBt{s}") for s in range(SLOTS)]

        dma_sem = ctx.enter_context(nc.semaphore("dma_sem"))
        act_sem = ctx.enter_context(nc.semaphore("act_sem"))
        dve_sem = ctx.enter_context(nc.semaphore("dve_sem"))
        block = ctx.enter_context(nc.Block())

        @block.sync
        def _(sync):
            # X[cc] partition p holds x[4*cc + p//32, :] (source AP
            # replicates each channel row 32x via a step-0 dim)
            for cc in range(2):
                src = bass.AP(x_d.tensor, 4 * cc * L,
                              [[L, 4], [0, 32], [1, L]])
                sync.dma_start(X[cc].ap(), src).then_inc(dma_sem, 16)
            ksrc = bass.AP(k_d.tensor, 0, [[0, 4], [K, F], [1, K]])
            sync.dma_start(negK.ap(), ksrc).then_inc(dma_sem, 16)
            for u, (cc, wc) in enumerate(UNITS_R):
                s = u % SLOTS
                sync.wait_ge(act_sem, act_through_copy(u))
                sync.dma_start(
                    out_d[128 * cc:128 * (cc + 1),
                          NWC * wc:NWC * (wc + 1)],
                    OB[s].ap()).then_inc(dma_sem, 16)

        @block.vector
        def _(vector):
            # init: m0 = BIG with 0 at each segment's cell j=0 (offset 1);
            # M BIG (so m[0] defined); D separator columns BIG
            vector.memset(m0.ap(), BIG).then_inc(dve_sem, 1)
            m0_seg = m0.ap().rearrange("p (w s) -> p w s", s=SEG)
            vector.memset(m0_seg[:, :, 1], 0.0).then_inc(dve_sem, 1)
            for s in range(SLOTS):
                vector.memset(M[s].ap(), BIG).then_inc(dve_sem, 1)
                for i in range(2):
                    d_seg = D[s][i].ap().rearrange("p (w s) -> p w s", s=SEG)
                    vector.memset(d_seg[:, :, 0], BIG).then_inc(dve_sem, 1)
            act_waited = 0
            for u, (cc, wc) in enumerate(UNITS_R):
                s = u % SLOTS
                cur = 0
                for i in range(K):
                    if i == 0:
                        m_ap = m0.ap()
                    else:
                        prev = S[s][cur].ap()
                        vector.tensor_tensor(
                            M[s].ap()[:, 1:], prev[:, 1:], prev[:, :-1],
                            mybir.AluOpType.min).then_inc(dve_sem, 1)
                        m_ap = M[s].ap()
                        cur ^= 1
                    need = act_through_square(u, i)
                    if need > act_waited:
                        vector.wait_ge(act_sem, need)
                        act_waited = need
                    vector.tensor_tensor_scan(
                        S[s][cur].ap(), m_ap, D[s][i % 2].ap(), float(BIG),
                        op0=mybir.AluOpType.min,
                        op1=mybir.AluOpType.add).then_inc(dve_sem, 1)

        @block.scalar
        def _(scalar):
            scalar.wait_ge(dma_sem, 48)
            dve_waited = 0
            dma_waited = 48
            for u, (cc, wc) in enumerate(UNITS_R):
                s = u % SLOTS
                xt = X[cc].ap()
                win = bass.AP(xt.tensor, xt.offset + 5 * NWC * wc,
                              [list(xt.ap[0]), [5, NWC], [1, PROC]])
                for i in range(K):
                    # WAR: D[s][i%2] was last read by an earlier scan
                    if i >= 2:
                        need = dve_through_scan(u, i - 2)
                    elif u >= SLOTS:
                        need = dve_through_scan(u - SLOTS, 8 + i)
                    else:
                        need = 0
                    if need > dve_waited:
                        scalar.wait_ge(dve_sem, need)
                        dve_waited = need
                    d_seg = D[s][i % 2].ap().rearrange(
                        "p (w s) -> p w s", s=SEG)
                    scalar.activation(
                        d_seg[:, :, 1:], win,
                        mybir.ActivationFunctionType.Square,
                        bias=negK.ap()[:, i:i + 1],
                        scale=1.0).then_inc(act_sem, 1)
                # extract: cell j=19 lives at segment offset 20; final row
                # (i=9, odd) lands in S[s][1]
                need = dve_through_scan(u, K - 1)
                if need > dve_waited:
                    scalar.wait_ge(dve_sem, need)
                    dve_waited = need
                if u >= SLOTS:
                    dneed = dma_through_out(u - SLOTS)
                    if dneed > dma_waited:
                        scalar.wait_ge(dma_sem, dneed)
                        dma_waited = dneed
                s_seg = S[s][1].ap().rearrange("p (w s) -> p w s", s=SEG)
                scalar.copy(OB[s].ap(), s_seg[:, :, SEG - 1]).then_inc(
                    act_sem, 1)
    return nc


_NC_CACHE = None


def kernel(x: np.ndarray, kernels: np.ndarray) -> np.ndarray:
    global _NC_CACHE
    if _NC_CACHE is None:
        _NC_CACHE = _build_nc()
    nc = _NC_CACHE
    x = np.ascontiguousarray(x, dtype=np.float32)
    negk = np.ascontiguousarray(-np.asarray(kernels, dtype=np.float32))
    in_maps = [{"x": x[b], "negk": negk} for b in range(B)]
    res = run_bass_kernel_spmd(nc, in_maps, core_ids=list(range(B)))
    out = np.stack([res.results[b]["out"] for b in range(B)], axis=0)
    return out[:, :, :NW]


# revision 17
# speedup vs baseline: 1.0304x; 1.0304x over previous
"""DTW layer (short kernel) Trainium2 Bass kernel.

Problem: x (B=8, C=8, L=4096) f32, kernels (F=32, K=10) f32.
For each (b, c, f, w): DTW cost between kernels[f] (len 10) and window
x[b, c, 5w : 5w+20], for w in [0, 815). Output (B, C*F, 815) f32.

Sharding: data-parallel over batch — core b computes batch b entirely
(C*F = 256 (c,f) combos = 2 partition chunks of 128).

Algorithm (per core): the DTW row recurrence
    row_i[j] = D[i,j] + min(row_i[j-1], row_{i-1}[j], row_{i-1}[j-1])
is computed for 128 (c,f) combos at once (partition dim) and a chunk of
windows laid out along the free dim as [w, 21] segments (1 separator +
20 cells).  Per row:
  - ACT computes local costs D[w, 1+j] = (x[5w+j] - k_i)^2 via
    activation(Square, bias=-k_i) with an overlapping strided input AP.
  - DVE computes m[t] = min(S_prev[t], S_prev[t-1]) (one shifted min).
  - DVE tensor_tensor_scan: state = min(m[t], state) + D[t] computes the
    whole row for all windows in one instruction.  A BIG value in the
    separator column of D forces the carry to BIG between windows, which
    the min against m (= prev row values) then discards — resetting the
    recurrence at each window boundary.

Raw bass (no Tile framework): this toolchain's walrus codegen allows at
most 2 embedded sync-waits per instruction and rejects Tile's tail
drain, so engines are programmed directly with standalone wait_ge
instructions and three semaphores (dma/act/dve).
"""

from contextlib import ExitStack

import numpy as np

import concourse.bass as bass
import concourse.mybir as mybir
from concourse.bass_utils import run_bass_kernel_spmd

# Problem constants (hardcoded per harness contract)
B, C, L = 8, 8, 4096
F, K = 32, 10
PROC, STEP = 20, 5
NW = 815          # windows actually computed == chan_outlen
SEG = PROC + 1    # 1 separator + 20 cells
NWC = 136         # windows per chunk; 6 chunks = 816 >= 815
NCHUNK = 6
TFREE = NWC * SEG # 2856 scan length
BIG = 1e30
SLOTS = 2
UNITS = [(cc, wc) for cc in range(2) for wc in range(NCHUNK)]

F32 = mybir.dt.float32


def _build_nc(reps: int = 1, small_m: bool = False,
              small_scan: bool = False, small_act: bool = False
              ) -> bass.Bass:
    # detect_race_conditions=False: CoreSim's detector does not model
    # same-engine program order, which this kernel relies on throughout.
    # reps > 1 replicates the whole schedule (for slope-based timing).
    nc = bass.Bass("TRN2", debug=False, detect_race_conditions=False)
    x_d = nc.dram_tensor("x", [C, L], F32, kind="ExternalInput").ap()
    k_d = nc.dram_tensor("negk", [F, K], F32, kind="ExternalInput").ap()
    out_d = nc.dram_tensor("out", [C * F, NWC * NCHUNK], F32,
                           kind="ExternalOutput").ap()

    UNITS_R = UNITS * reps

    # --- semaphore bookkeeping (python-side counts) ---
    # DVE op order: 8 init memsets, then per unit: scan0, (m,scan)x9 = 19
    def dve_through_scan(u, i):
        return 8 + 19 * u + (1 if i == 0 else 2 * i + 1)

    # ACT op order: per unit: 10 squares + 1 extract copy
    def act_through_square(u, i):
        return 11 * u + i + 1

    def act_through_copy(u):
        return 11 * (u + 1)

    def dma_through_out(u):  # 3 init DMAs then one out-DMA per unit
        return 16 * (4 + u)

    with ExitStack() as ctx:
        sb = lambda shape, name: ctx.enter_context(
            nc.sbuf_tensor(name, shape, F32))
        X = [sb([128, L], f"Xt{cc}") for cc in range(2)]
        negK = sb([128, K], "negKt")
        m0 = sb([128, TFREE], "m0t")
        S = [[sb([128, TFREE], f"St{s}_{i}") for i in range(2)]
             for s in range(SLOTS)]
        M = [sb([128, TFREE], f"Mt{s}") for s in range(SLOTS)]
        D = [[sb([128, TFREE], f"Dt{s}_{i}") for i in range(2)]
             for s in range(SLOTS)]
        OB = [sb([128, NWC], f"OBt{s}") for s in range(SLOTS)]

        dma_sem = ctx.enter_context(nc.semaphore("dma_sem"))
        act_sem = ctx.enter_context(nc.semaphore("act_sem"))
        dve_sem = ctx.enter_context(nc.semaphore("dve_sem"))
        block = ctx.enter_context(nc.Block())

        @block.sync
        def _(sync):
            # X[cc] partition p holds x[4*cc + p//32, :] (source AP
            # replicates each channel row 32x via a step-0 dim)
            for cc in range(2):
                src = bass.AP(x_d.tensor, 4 * cc * L,
                              [[L, 4], [0, 32], [1, L]])
                sync.dma_start(X[cc].ap(), src).then_inc(dma_sem, 16)
            ksrc = bass.AP(k_d.tensor, 0, [[0, 4], [K, F], [1, K]])
            sync.dma_start(negK.ap(), ksrc).then_inc(dma_sem, 16)
            for u, (cc, wc) in enumerate(UNITS_R):
                s = u % SLOTS
                sync.wait_ge(act_sem, act_through_copy(u))
                sync.dma_start(
                    out_d[128 * cc:128 * (cc + 1),
                          NWC * wc:NWC * (wc + 1)],
                    OB[s].ap()).then_inc(dma_sem, 16)

        @block.vector
        def _(vector):
            # init: m0 = BIG with 0 at each segment's cell j=0 (offset 1);
            # M BIG (so m[0] defined); D separator columns BIG
            vector.memset(m0.ap(), BIG).then_inc(dve_sem, 1)
            m0_seg = m0.ap().rearrange("p (w s) -> p w s", s=SEG)
            vector.memset(m0_seg[:, :, 1], 0.0).then_inc(dve_sem, 1)
            for s in range(SLOTS):
                vector.memset(M[s].ap(), BIG).then_inc(dve_sem, 1)
                for i in range(2):
                    d_seg = D[s][i].ap().rearrange("p (w s) -> p w s", s=SEG)
                    vector.memset(d_seg[:, :, 0], BIG).then_inc(dve_sem, 1)
            act_waited = 0
            for u, (cc, wc) in enumerate(UNITS_R):
                s = u % SLOTS
                cur = 0
                for i in range(K):
                    if i == 0:
                        m_ap = m0.ap()
                    else:
                        prev = S[s][cur].ap()
                        if small_m:
                            vector.tensor_tensor(
                                M[s].ap()[:, 1:5], prev[:, 1:5], prev[:, :4],
                                mybir.AluOpType.min).then_inc(dve_sem, 1)
                        else:
                            vector.tensor_tensor(
                                M[s].ap()[:, 1:], prev[:, 1:], prev[:, :-1],
                                mybir.AluOpType.min).then_inc(dve_sem, 1)
                        m_ap = M[s].ap()
                        cur ^= 1
                    need = act_through_square(u, i)
                    if need > act_waited:
                        vector.wait_ge(act_sem, need)
                        act_waited = need
                    if small_scan:
                        vector.tensor_tensor_scan(
                            S[s][cur].ap()[:, :4], m_ap[:, :4],
                            D[s][i % 2].ap()[:, :4], float(BIG),
                            op0=mybir.AluOpType.min,
                            op1=mybir.AluOpType.add).then_inc(dve_sem, 1)
                    else:
                        vector.tensor_tensor_scan(
                            S[s][cur].ap(), m_ap, D[s][i % 2].ap(), float(BIG),
                            op0=mybir.AluOpType.min,
                            op1=mybir.AluOpType.add).then_inc(dve_sem, 1)

        @block.scalar
        def _(scalar):
            scalar.wait_ge(dma_sem, 48)
            dve_waited = 0
            dma_waited = 48
            for u, (cc, wc) in enumerate(UNITS_R):
                s = u % SLOTS
                xt = X[cc].ap()
                win = bass.AP(xt.tensor, xt.offset + 5 * NWC * wc,
                              [list(xt.ap[0]), [5, NWC], [1, PROC]])
                for i in range(K):
                    # WAR: D[s][i%2] was last read by an earlier scan
                    if i >= 2:
                        need = dve_through_scan(u, i - 2)
                    elif u >= SLOTS:
                        need = dve_through_scan(u - SLOTS, 8 + i)
                    else:
                        need = 0
                    if need > dve_waited:
                        scalar.wait_ge(dve_sem, need)
                        dve_waited = need
                    d_seg = D[s][i % 2].ap().rearrange(
                        "p (w s) -> p w s", s=SEG)
                    if small_act:
                        scalar.activation(
                            d_seg[:, :1, 1:], win[:, :1, :],
                            mybir.ActivationFunctionType.Square,
                            bias=negK.ap()[:, i:i + 1],
                            scale=1.0).then_inc(act_sem, 1)
                    else:
                        scalar.activation(
                            d_seg[:, :, 1:], win,
                            mybir.ActivationFunctionType.Square,
                            bias=negK.ap()[:, i:i + 1],
                            scale=1.0).then_inc(act_sem, 1)
                # extract: cell j=19 lives at segment offset 20; final row
                # (i=9, odd) lands in S[s][1]
                need = dve_through_scan(u, K - 1)
                if need > dve_waited:
                    scalar.wait_ge(dve_sem, need)
                    dve_waited = need
                if u >= SLOTS:
                    dneed = dma_through_out(u - SLOTS)
                    if dneed > dma_waited:
                        scalar.wait_ge(dma_sem, dneed)
                        dma_waited = dneed
                s_seg = S[s][1].ap().rearrange("p (w s) -> p w s", s=SEG)
                scalar.copy(OB[s].ap(), s_seg[:, :, SEG - 1]).then_inc(
                    act_sem, 1)
    return nc


_NC_CACHE = None


def kernel(x: np.ndarray, kernels: np.ndarray) -> np.ndarray:
    global _NC_CACHE
    if _NC_CACHE is None:
        _NC_CACHE = _build_nc()
    nc = _NC_CACHE
    x = np.ascontiguousarray(x, dtype=np.float32)
    negk = np.ascontiguousarray(-np.asarray(kernels, dtype=np.float32))
    in_maps = [{"x": x[b], "negk": negk} for b in range(B)]
    res = run_bass_kernel_spmd(nc, in_maps, core_ids=list(range(B)))
    out = np.stack([res.results[b]["out"] for b in range(B)], axis=0)
    return out[:, :, :NW]


# revision 23
# speedup vs baseline: 1.2274x; 1.1912x over previous
"""DTW layer (short kernel) Trainium2 Bass kernel.

Problem: x (B=8, C=8, L=4096) f32, kernels (F=32, K=10) f32.
For each (b, c, f, w): DTW cost between kernels[f] (len 10) and window
x[b, c, 5w : 5w+20], for w in [0, 815). Output (B, C*F, 815) f32.

Sharding: data-parallel over batch — core b computes batch b entirely
(C*F = 256 (c,f) combos = 2 partition chunks of 128).

Algorithm (per core): the DTW row recurrence
    row_i[j] = D[i,j] + min(row_i[j-1], row_{i-1}[j], row_{i-1}[j-1])
is computed for 128 (c,f) combos at once (partition dim) and a chunk of
windows laid out along the free dim as [w, 21] segments (1 separator +
20 cells).  Per row:
  - ACT computes local costs D[w, 1+j] = (x[5w+j] - k_i)^2 via
    activation(Square, bias=-k_i) with an overlapping strided input AP.
  - GPSIMD (or DVE) computes m[t] = min(S_prev[t], S_prev[t-1]).
  - DVE tensor_tensor_scan: state = min(m[t], state) + D[t] computes the
    whole row for all windows in one instruction.  A BIG value in the
    separator column of D forces the carry to BIG between windows, which
    the min against m (= prev row values) then discards — resetting the
    recurrence at each window boundary.

Raw bass (no Tile framework): this toolchain's walrus codegen allows at
most 2 embedded sync-waits per instruction and rejects Tile's tail
drain, so engines are programmed directly with standalone wait_ge
instructions and per-engine semaphores.
"""

from contextlib import ExitStack

import numpy as np

import concourse.bass as bass
import concourse.mybir as mybir
from concourse.bass_utils import run_bass_kernel_spmd

# Problem constants (hardcoded per harness contract)
B, C, L = 8, 8, 4096
F, K = 32, 10
PROC, STEP = 20, 5
NW = 815          # windows actually computed == chan_outlen
SEG = PROC + 1    # 1 separator + 20 cells
NWC = 136         # windows per chunk; 6 chunks = 816 >= 815
NCHUNK = 6
TFREE = NWC * SEG # 2856 scan length
BIG = 1e30
SLOTS = 2
UNITS = [(cc, wc) for cc in range(2) for wc in range(NCHUNK)]

F32 = mybir.dt.float32
F16 = mybir.dt.float16


def _build_nc(reps: int = 1, gp_m: bool = False, dt16: bool = False,
              small_m: bool = False, small_scan: bool = False,
              small_act: bool = False) -> bass.Bass:
    """gp_m: run the shifted-min on GPSIMD (off DVE's critical path).
    dt16: keep state/cost tiles in bf16 (DVE 2x mode candidates).
    small_*: shrink one op class to 4 elements (timing attribution).
    reps > 1 replicates the schedule (slope-based timing)."""
    # detect_race_conditions=False: CoreSim's detector does not model
    # same-engine program order, which this kernel relies on throughout.
    nc = bass.Bass("TRN2", debug=False, detect_race_conditions=False)
    x_d = nc.dram_tensor("x", [C, L], F32, kind="ExternalInput").ap()
    k_d = nc.dram_tensor("negk", [F, K], F32, kind="ExternalInput").ap()
    out_d = nc.dram_tensor("out", [C * F, NWC * NCHUNK], F32,
                           kind="ExternalOutput").ap()

    UNITS_R = UNITS * reps
    SDT = F16 if dt16 else F32
    big = 30000.0 if dt16 else BIG

    # --- semaphore bookkeeping (python-side op counts) ---
    # DVE order: 8 init memsets, then per unit 10 scans (gp_m) or
    # scan0 + (m, scan) x 9 (no gp_m)
    def dve_through_scan(u, i):
        if gp_m:
            return 8 + 10 * u + i + 1
        return 8 + 19 * u + (1 if i == 0 else 2 * i + 1)

    def gp_through_m(u, i):  # 9 m-ops per unit, i in 1..9
        return 9 * u + i

    # ACT order: per unit 10 squares + 1 extract copy
    def act_through_square(u, i):
        return 11 * u + i + 1

    def act_through_copy(u):
        return 11 * (u + 1)

    def dma_through_out(u):  # 3 init DMAs then one out-DMA per unit
        return 16 * (4 + u)

    with ExitStack() as ctx:
        sb = lambda shape, name, dt: ctx.enter_context(
            nc.sbuf_tensor(name, shape, dt))
        X = [sb([128, L], f"Xt{cc}", F32) for cc in range(2)]
        negK = sb([128, K], "negKt", F32)
        m0 = sb([128, TFREE], "m0t", SDT)
        S = [[sb([128, TFREE], f"St{s}_{i}", SDT) for i in range(2)]
             for s in range(SLOTS)]
        M = [sb([128, TFREE], f"Mt{s}", SDT) for s in range(SLOTS)]
        D = [[sb([128, TFREE], f"Dt{s}_{i}", SDT) for i in range(2)]
             for s in range(SLOTS)]
        OB = [sb([128, NWC], f"OBt{s}", F32) for s in range(SLOTS)]

        dma_sem = ctx.enter_context(nc.semaphore("dma_sem"))
        act_sem = ctx.enter_context(nc.semaphore("act_sem"))
        dve_sem = ctx.enter_context(nc.semaphore("dve_sem"))
        gp_sem = ctx.enter_context(nc.semaphore("gp_sem"))
        block = ctx.enter_context(nc.Block())

        @block.sync
        def _(sync):
            # X[cc] partition p holds x[4*cc + p//32, :] (source AP
            # replicates each channel row 32x via a step-0 dim)
            for cc in range(2):
                src = bass.AP(x_d.tensor, 4 * cc * L,
                              [[L, 4], [0, 32], [1, L]])
                sync.dma_start(X[cc].ap(), src).then_inc(dma_sem, 16)
            ksrc = bass.AP(k_d.tensor, 0, [[0, 4], [K, F], [1, K]])
            sync.dma_start(negK.ap(), ksrc).then_inc(dma_sem, 16)
            for u, (cc, wc) in enumerate(UNITS_R):
                s = u % SLOTS
                sync.wait_ge(act_sem, act_through_copy(u))
                sync.dma_start(
                    out_d[128 * cc:128 * (cc + 1),
                          NWC * wc:NWC * (wc + 1)],
                    OB[s].ap()).then_inc(dma_sem, 16)

        def emit_m(eng, u, s, prev):
            if small_m:
                return eng.tensor_tensor(M[s].ap()[:, 1:5], prev[:, 1:5],
                                         prev[:, :4],
                                         mybir.AluOpType.min)
            return eng.tensor_tensor(M[s].ap()[:, 1:], prev[:, 1:],
                                     prev[:, :-1], mybir.AluOpType.min)

        if gp_m:
            @block.gpsimd
            def _(gpsimd):
                dve_waited = 0
                for u, (cc, wc) in enumerate(UNITS_R):
                    s = u % SLOTS
                    for i in range(1, K):
                        need = dve_through_scan(u, i - 1)
                        if need > dve_waited:
                            gpsimd.wait_ge(dve_sem, need)
                            dve_waited = need
                        emit_m(gpsimd, u, s,
                               S[s][(i - 1) % 2].ap()).then_inc(gp_sem, 1)

        @block.vector
        def _(vector):
            # init: m0 = BIG with 0 at each segment's cell j=0 (offset 1);
            # M BIG (so m[0] defined); D separator columns BIG
            vector.memset(m0.ap(), big).then_inc(dve_sem, 1)
            m0_seg = m0.ap().rearrange("p (w s) -> p w s", s=SEG)
            vector.memset(m0_seg[:, :, 1], 0.0).then_inc(dve_sem, 1)
            for s in range(SLOTS):
                vector.memset(M[s].ap(), big).then_inc(dve_sem, 1)
                for i in range(2):
                    d_seg = D[s][i].ap().rearrange("p (w s) -> p w s", s=SEG)
                    vector.memset(d_seg[:, :, 0], big).then_inc(dve_sem, 1)
            act_waited = 0
            gp_waited = 0
            for u, (cc, wc) in enumerate(UNITS_R):
                s = u % SLOTS
                cur = 0
                for i in range(K):
                    if i == 0:
                        m_ap = m0.ap()
                    else:
                        if gp_m:
                            need = gp_through_m(u, i)
                            if need > gp_waited:
                                vector.wait_ge(gp_sem, need)
                                gp_waited = need
                        else:
                            emit_m(vector, u, s,
                                   S[s][cur].ap()).then_inc(dve_sem, 1)
                        m_ap = M[s].ap()
                        cur ^= 1
                    need = act_through_square(u, i)
                    if need > act_waited:
                        vector.wait_ge(act_sem, need)
                        act_waited = need
                    if small_scan:
                        vector.tensor_tensor_scan(
                            S[s][cur].ap()[:, :4], m_ap[:, :4],
                            D[s][i % 2].ap()[:, :4], float(big),
                            op0=mybir.AluOpType.min,
                            op1=mybir.AluOpType.add).then_inc(dve_sem, 1)
                    else:
                        vector.tensor_tensor_scan(
                            S[s][cur].ap(), m_ap, D[s][i % 2].ap(),
                            float(big),
                            op0=mybir.AluOpType.min,
                            op1=mybir.AluOpType.add).then_inc(dve_sem, 1)

        @block.scalar
        def _(scalar):
            scalar.wait_ge(dma_sem, 48)
            dve_waited = 0
            dma_waited = 48
            for u, (cc, wc) in enumerate(UNITS_R):
                s = u % SLOTS
                xt = X[cc].ap()
                win = bass.AP(xt.tensor, xt.offset + 5 * NWC * wc,
                              [list(xt.ap[0]), [5, NWC], [1, PROC]])
                for i in range(K):
                    # WAR: D[s][i%2] was last read by an earlier scan
                    if i >= 2:
                        need = dve_through_scan(u, i - 2)
                    elif u >= SLOTS:
                        need = dve_through_scan(u - SLOTS, 8 + i)
                    else:
                        need = 0
                    if need > dve_waited:
                        scalar.wait_ge(dve_sem, need)
                        dve_waited = need
                    d_seg = D[s][i % 2].ap().rearrange(
                        "p (w s) -> p w s", s=SEG)
                    if small_act:
                        scalar.activation(
                            d_seg[:, :1, 1:], win[:, :1, :],
                            mybir.ActivationFunctionType.Square,
                            bias=negK.ap()[:, i:i + 1],
                            scale=1.0).then_inc(act_sem, 1)
                    else:
                        scalar.activation(
                            d_seg[:, :, 1:], win,
                            mybir.ActivationFunctionType.Square,
                            bias=negK.ap()[:, i:i + 1],
                            scale=1.0).then_inc(act_sem, 1)
                # extract: cell j=19 lives at segment offset 20; final row
                # (i=9, odd) lands in S[s][1]
                need = dve_through_scan(u, K - 1)
                if need > dve_waited:
                    scalar.wait_ge(dve_sem, need)
                    dve_waited = need
                if u >= SLOTS:
                    dneed = dma_through_out(u - SLOTS)
                    if dneed > dma_waited:
                        scalar.wait_ge(dma_sem, dneed)
                        dma_waited = dneed
                s_seg = S[s][1].ap().rearrange("p (w s) -> p w s", s=SEG)
                scalar.copy(OB[s].ap(), s_seg[:, :, SEG - 1]).then_inc(
                    act_sem, 1)
    return nc


_NC_CACHE = None


def kernel(x: np.ndarray, kernels: np.ndarray) -> np.ndarray:
    global _NC_CACHE
    if _NC_CACHE is None:
        _NC_CACHE = _build_nc()
    nc = _NC_CACHE
    x = np.ascontiguousarray(x, dtype=np.float32)
    negk = np.ascontiguousarray(-np.asarray(kernels, dtype=np.float32))
    in_maps = [{"x": x[b], "negk": negk} for b in range(B)]
    res = run_bass_kernel_spmd(nc, in_maps, core_ids=list(range(B)))
    out = np.stack([res.results[b]["out"] for b in range(B)], axis=0)
    return out[:, :, :NW]
